# revision 86
# baseline (speedup 1.0000x reference)
"""Multi-head attention (B=8, N=1024, C=1024, H=16) on 8 Trainium2 NeuronCores.

Sharding: pure data-parallel — one batch element per core, weights replicated,
no collectives.

v2: all matmuls in bf16 (fp32 matmuls are power-throttled to ~half rate on
TRN2; bf16 runs the PE at full 2.4 GHz). Everything stays SBUF-resident (no
DRAM bounce for qkT). Fused per-head-pair schedule so the ACT engine (exp)
overlaps the projection matmuls. Accumulation is fp32 in PSUM throughout.

Per-core algorithm:
  qk proj   qkT[d, n] = wqkT[c, d].T @ xT[c, n] per head pair, cast bf16.
  v proj    v[m, d] natural layout, interleaved per m-tile as [m, 16*(64+1)]
            with a ones column per head (PV then emits softmax denominators
            for free in PSUM row 64).
  attention per unit (head, nt-half):
            S^T[m, n] = k.T @ q      (keys on partitions, K=64 row-packed)
            E = exp(SCALE * S^T)     (ACT, PSUM -> SBUF bf16)
            U[65, n] = v_aug.T @ E   (row 64 = denominator)
            rden = reciprocal(U[64]) (DVE, fp32)  -> cast bf16
            bc[128, n] = ones_bd.T @ rden   (PE broadcast across partitions)
            aot[c, n] = U[0:64] * bc        (Pool engine, writes bf16)
  proj      out[n, d] = aot[c, n].T @ wpT[c, d] + bias (Pool add), DMA out.
"""

import sys

if "/opt/trn_rl_repo" not in sys.path:
    sys.path.insert(0, "/opt/trn_rl_repo")

from contextlib import ExitStack

import numpy as np

import concourse.bass as bass
import concourse.mybir as mybir
from concourse import bacc
import concourse.tile as tile
from concourse import bass_utils

B, N, C, H = 8, 1024, 1024, 16
HD = C // H          # 64
NP = H // 2          # 8 head pairs
SCALE = HD ** -0.5   # 0.125
P = 128              # SBUF partitions
NT = 512             # psum-bank moving tile
NCH = C // P         # 8 contraction chunks over channels
NMT = N // P         # 8 token tiles of 128
NNT = N // NT        # 2 token tiles of 512
F32 = mybir.dt.float32
BF16 = mybir.dt.bfloat16
EXP = mybir.ActivationFunctionType.Exp


def build_module():
    import os
    _SKIP = set(filter(None, os.environ.get("K_SKIP", "").split(",")))
    nc = bacc.Bacc("TRN2", target_bir_lowering=False, debug=False, num_devices=B)

    xT = nc.dram_tensor("xT", [C, N], BF16, kind="ExternalInput").ap()
    wqkT = nc.dram_tensor("wqkT", [C, 2 * C], BF16, kind="ExternalInput").ap()
    # pair-0's q/k weight columns, duplicated for a fast startup path
    wqk0 = nc.dram_tensor("wqk0", [C, 2 * P], BF16, kind="ExternalInput").ap()
    wvT = nc.dram_tensor("wvT", [C, C], BF16, kind="ExternalInput").ap()
    wpT = nc.dram_tensor("wpT", [C, C], BF16, kind="ExternalInput").ap()
    bias = nc.dram_tensor("bias_bc", [P, C], F32, kind="ExternalInput").ap()
    ones_col = nc.dram_tensor("ones_col", [P, H], BF16, kind="ExternalInput").ap()
    ones_bd = nc.dram_tensor("ones_bd", [P, P], BF16, kind="ExternalInput").ap()
    out = nc.dram_tensor("out", [N, C], F32, kind="ExternalOutput").ap()
    _DBG = os.environ.get("K_DEBUG", "")
    dbg = {}
    if _DBG:
        dbg["aot"] = nc.dram_tensor("dbg_aot", [C, N], BF16, kind="ExternalOutput").ap()
        dbg["qk"] = nc.dram_tensor("dbg_qk", [2 * P, N], BF16, kind="ExternalOutput").ap()
        dbg["v"] = nc.dram_tensor("dbg_v", [P, H * (HD + 1)], BF16, kind="ExternalOutput").ap()
        dbg["e"] = nc.dram_tensor("dbg_e", [P, N * NMT // 2], BF16, kind="ExternalOutput").ap()
        dbg["rb"] = nc.dram_tensor("dbg_rb", [P, NT], BF16, kind="ExternalOutput").ap()
        dbg["rc"] = nc.dram_tensor("dbg_rc", [P, NT], F32, kind="ExternalOutput").ap()

    with tile.TileContext(nc) as tc, ExitStack() as ctx:
        xt_pool = ctx.enter_context(tc.tile_pool(name="xt", bufs=NCH))
        wqk_pool = ctx.enter_context(tc.tile_pool(name="wqk", bufs=NCH))
        wv_pool = ctx.enter_context(tc.tile_pool(name="wv", bufs=NCH))
        wp_pool = ctx.enter_context(tc.tile_pool(name="wp", bufs=NCH))
        qk_pool = ctx.enter_context(tc.tile_pool(name="qk", bufs=3))
        vsb_pool = ctx.enter_context(tc.tile_pool(name="vsb", bufs=1))
        e_pool = ctx.enter_context(tc.tile_pool(name="e", bufs=5))
        aot_pool = ctx.enter_context(tc.tile_pool(name="aot", bufs=1))
        one_pool = ctx.enter_context(tc.tile_pool(name="one", bufs=1))
        rden_pool = ctx.enter_context(tc.tile_pool(name="rden", bufs=2))
        osb_pool = ctx.enter_context(tc.tile_pool(name="osb", bufs=2))
        ps_pool = ctx.enter_context(tc.tile_pool(name="ps", bufs=2, space="PSUM"))
        ps2_pool = ctx.enter_context(tc.tile_pool(name="ps2", bufs=1, space="PSUM"))
        pu_pool = ctx.enter_context(tc.tile_pool(name="pu", bufs=3, space="PSUM"))

        # ---------- input loads ----------
        # pair-0 weight slices ride interleaved with x so qk(0) is paced by
        # the x tiles, not the full 4MB wqk load
        # The pair-0 weight fast path (wqk0) shaved ~6us of startup but showed
        # rare first-run corruption on hardware (cold-DMA race); disabled.
        _NOWQK0 = not os.environ.get("K_WQK0")
        xts, wvs, wqks, wqk0s = [], [], [], []
        for t in range(NCH):
            xt_t = xt_pool.tile([P, N], BF16, tag="xt", name=f"xt{t}")
            nc.sync.dma_start(xt_t, xT[t * P : (t + 1) * P, :])
            xts.append(xt_t)
            if not _NOWQK0:
                w0_t = wqk_pool.tile([P, 2 * P], BF16, tag="wqk0", name=f"wqk0_{t}")
                nc.sync.dma_start(w0_t, wqk0[t * P : (t + 1) * P, :])
                wqk0s.append(w0_t)
        for t in range(NCH):
            wqk_t = wqk_pool.tile([P, 2 * C], BF16, tag="wqk", name=f"wqk{t}")
            nc.sync.dma_start(wqk_t, wqkT[t * P : (t + 1) * P, :])
            wqks.append(wqk_t)
        for t in range(NCH):
            wv_t = wv_pool.tile([P, C], BF16, tag="wv", name=f"wv{t}")
            nc.sync.dma_start(wv_t, wvT[t * P : (t + 1) * P, :])
            wvs.append(wv_t)
        bias_sb = one_pool.tile([P, C], F32, tag="bias", name="bias_sb")
        nc.sync.dma_start(bias_sb, bias)
        onesbd_sb = one_pool.tile([P, P], BF16, tag="obd", name="onesbd_sb")
        nc.sync.dma_start(onesbd_sb, ones_bd)

        # persistent zero-padded k tiles and denominator-broadcast sources:
        # zeroed once here; later writers only touch the data quadrants /
        # rows 0 and 64, so the zero regions stay valid across reuse
        kpads = []
        for i in range(3):
            kp_t = qk_pool.tile([P, 2 * N], BF16, tag="kpad", name=f"kpad{i}")
            nc.gpsimd.memset(kp_t, 0.0)
            kpads.append(kp_t)
        rbs = []
        for i in range(2):
            rb_t = rden_pool.tile([P, NT], BF16, tag="rb", name=f"rb{i}")
            nc.gpsimd.memset(rb_t, 0.0)
            rbs.append(rb_t)

        # v tiles (natural layout + ones cols), attention-out accumulators
        vsb = []
        for mt in range(NMT):
            v_t = vsb_pool.tile([P, H * (HD + 1)], BF16, tag=f"v{mt}", name=f"v{mt}")
            nc.sync.dma_start(
                v_t.rearrange("p (h w) -> p h w", w=HD + 1)[:, :, HD : HD + 1], ones_col
            )
            vsb.append(v_t)
        aot = []
        for t in range(NCH):
            a_t = aot_pool.tile([P, N], BF16, tag=f"aot{t}", name=f"aot{t}")
            aot.append(a_t)

        # ---------- phase emitters ----------
        def emit_v_mt(mt):
            """v projection for one m-tile: psum [128, 1024] = both d halves."""
            ps_v = ps_pool.tile([P, N], F32, tag="ps", name=f"psv{mt}")
            for ck in range(NCH):
                for hv in range(2):
                    nc.tensor.matmul(
                        ps_v[:, hv * NT : (hv + 1) * NT],
                        lhsT=xts[ck][:, mt * P : (mt + 1) * P],
                        rhs=wvs[ck][:, hv * NT : (hv + 1) * NT],
                        start=(ck == 0),
                        stop=(ck == NCH - 1),
                    )
            dst = vsb[mt].rearrange("p (h w) -> p h w", w=HD + 1)[:, :, 0:HD]
            nc.vector.tensor_copy(dst, ps_v.rearrange("p (h w) -> p h w", w=HD))

        def emit_qk_chunks(p):
            """qk projection for head pair p, as a generator yielding after
            each (tensor, nt-half) chunk so the chunks can be spread across
            the previous pair's unit slots (keeps the PE fed while the ACT
            engine drains exps). q -> bf16 [128, 1024]; k -> a zero-padded
            [128, 2048] (head A in rows 0:64 of cols 0:1024, head B in rows
            64:128 of cols 1024:2048, zeros elsewhere) so the S matmuls
            contract over K=128 — K=64 matmuls run ~1.8x slower on the PE."""
            q_sb = qk_pool.tile([P, N], BF16, tag="qk", name=f"qk{p}")
            kpad = kpads[p % 3]

            def gen():
                for which in range(2):  # 0 = q rows, 1 = k rows
                    dlo = which * C + p * P
                    sb = q_sb if which == 0 else kpad
                    for nt in range(NNT):
                        ps_qk = ps2_pool.tile(
                            [P, NT], F32, tag="ps2", name=f"psqk{p}_{which}_{nt}"
                        )
                        for ck in range(NCH):
                            lhsT = (
                                wqk0s[ck][:, which * P : (which + 1) * P]
                                if p == 0 and wqk0s
                                else wqks[ck][:, dlo : dlo + P]
                            )
                            nc.tensor.matmul(
                                ps_qk,
                                lhsT=lhsT,
                                rhs=xts[ck][:, nt * NT : (nt + 1) * NT],
                                start=(ck == 0),
                                stop=(ck == NCH - 1),
                            )
                        if which == 0:
                            nc.vector.tensor_copy(sb[:, nt * NT : (nt + 1) * NT], ps_qk)
                        else:
                            nc.vector.tensor_copy(
                                sb[0:HD, nt * NT : (nt + 1) * NT], ps_qk[0:HD, :]
                            )
                            nc.vector.tensor_copy(
                                sb[HD:P, N + nt * NT : N + (nt + 1) * NT],
                                ps_qk[HD:P, :],
                            )
                        yield

            return [q_sb, kpad], gen()

        def emit_qk(p):
            tiles, g = emit_qk_chunks(p)
            for _ in g:
                pass
            return tiles

        def emit_s_exp(u, pv_item=None, norm_cb=None):
            """S^T matmuls + exp for one unit (pair p, head-slot j, nt).
            The S block is split in two halves with the previous unit's PV
            block (and the pending normalize) between them, so the PE has a
            block of independent matmuls to chew while the exps of the first
            half drain their psum tiles. Contracts over all 128 partitions;
            the kpad zero rows mask out the other head."""
            (p, j, nt, qt, kt) = u
            e_t = e_pool.tile([P, N * NMT // 2], BF16, tag="e", name=f"e{p}_{j}_{nt}")

            def s_half(g0):
                for g in range(g0, g0 + 2):  # two m-chunks per psum tile
                    ps_s = ps_pool.tile([P, N], F32, tag="ps", name=f"pss{p}_{j}_{nt}_{g}")
                    for half in range(2):
                        mc = 2 * g + half
                        nc.tensor.matmul(
                            ps_s[:, half * NT : (half + 1) * NT],
                            lhsT=kt[:, j * N + mc * P : j * N + (mc + 1) * P],
                            rhs=qt[:, nt * NT : (nt + 1) * NT],
                            start=True,
                            stop=True,
                        )
                    nc.scalar.activation(
                        e_t[:, g * N : (g + 1) * N], ps_s, EXP, scale=SCALE
                    )

            s_half(0)
            s_half(2)
            flush_dens()
            if norm_cb is not None:
                norm_cb()
            if pv_item is not None:
                emit_pv(*pv_item)
            return e_t

        pair_nt_state = {}
        pending_norms = []
        pending_dens = []

        def flush_dens():
            while pending_dens:
                emit_pv_den(*pending_dens.pop(0))

        def emit_pv_mms(u, e_t):
            """Generator: PV matmuls two at a time (yield between batches),
            then the denominator ln/exp chain."""
            (p, j, nt, qt, kt) = u
            h = 2 * p + j
            ps_u = pu_pool.tile([HD + 1, NT], F32, tag="pu", name=f"psu{h}_{nt}")
            for mc in range(NMT):
                nc.tensor.matmul(
                    ps_u,
                    lhsT=vsb[mc][:, h * (HD + 1) : (h + 1) * (HD + 1)],
                    rhs=e_t[:, mc * NT : (mc + 1) * NT],
                    start=(mc == 0),
                    stop=(mc == NMT - 1),
                )
                if mc % 2 == 1 and mc < NMT - 1:
                    yield
            # defer the denominator ln/exp one slot (pu bufs=3 gives the
            # normalize chain the slack): ACT drains ready exps first instead
            # of idling at an Ln that waits on this PV's U row
            pending_dens.append((u, ps_u))
            yield

        def emit_pv(u, e_t):
            for _ in emit_pv_mms(u, e_t):
                pass

        def emit_pv_den(u, ps_u):
            """Denominator 1/den = exp(-ln(den)) on ACT (same act table
            as the attention exps -> no table switches; a DVE exact
            reciprocal was tried instead and regressed 48us — its 4us serial
            op blocks DVE's in-order queue ahead of the latency-critical qk
            casts). The broadcast matmul + normalize is queued and emitted a
            slot later so the PE never stalls in-order on the ACT chain."""
            (p, j, nt, qt, kt) = u
            key = (p, nt)
            if key not in pair_nt_state:
                rb = rbs[(2 * p + nt) % 2]
                lt = rden_pool.tile([1, 2 * NT], F32, tag="lt", name=f"lt{p}_{nt}")
                pair_nt_state[key] = {"rb": rb, "lt": lt, "us": []}
            st = pair_nt_state[key]
            LN = mybir.ActivationFunctionType.Ln
            nc.scalar.activation(
                st["lt"][:, j * NT : (j + 1) * NT], ps_u[HD : HD + 1, :], LN
            )
            nc.scalar.activation(
                st["rb"][j * HD : j * HD + 1, :],
                st["lt"][:, j * NT : (j + 1) * NT],
                EXP,
                scale=-1.0,
            )
            st["us"].append((j, ps_u))
            if len(st["us"]) == 2:
                pair_nt_state.pop(key)
                pending_norms.append((p, nt, st))

        def emit_norm():
            if not pending_norms:
                return None
            (p, nt, st) = pending_norms.pop(0)
            bc = ps2_pool.tile([P, NT], F32, tag="ps2", name=f"bc{p}_{nt}")
            nc.tensor.matmul(
                bc,
                lhsT=onesbd_sb[0 : HD + 1, :],
                rhs=st["rb"][0 : HD + 1, :],
                start=True,
                stop=True,
            )
            rbc = rden_pool.tile([HD, 2 * NT], F32, tag="rc", name=f"rc{p}_{nt}")
            nc.vector.tensor_copy(rbc[:, 0:NT], bc[0:HD, :])
            nc.vector.tensor_copy(rbc[:, NT : 2 * NT], bc[HD : 2 * HD, :])
            if _DBG and p == 0 and nt == 0:
                nc.sync.dma_start(dbg["rb"], st["rb"])
                nc.sync.dma_start(dbg["rc"][0:HD, :], rbc[:, 0:NT])
            for (jj, psu) in st["us"]:
                nc.vector.tensor_mul(
                    aot[p][jj * HD : (jj + 1) * HD, nt * NT : (nt + 1) * NT],
                    psu[0:HD, :],
                    rbc[:, jj * NT : (jj + 1) * NT],
                )
            return (p, nt)

        def emit_proj_nt2(nt2):
            ps_o = ps_pool.tile([P, C], F32, tag="ps", name=f"pso{nt2}")
            for ck in range(NCH):
                for dt in range(2):
                    nc.tensor.matmul(
                        ps_o[:, dt * NT : (dt + 1) * NT],
                        lhsT=aot[ck][:, nt2 * P : (nt2 + 1) * P],
                        rhs=wps[ck][:, dt * NT : (dt + 1) * NT],
                        start=(ck == 0),
                        stop=(ck == NCH - 1),
                    )
            o_sb = osb_pool.tile([P, C], F32, tag="osb", name=f"o{nt2}")
            nc.vector.tensor_add(o_sb, ps_o, bias_sb)
            nc.sync.dma_start(out[nt2 * P : (nt2 + 1) * P, :], o_sb)

        # ---------- fused schedule ----------
        wps = []
        if "pattn" in _SKIP:
            for t in range(NCH):
                nc.sync.dma_start(aot[t], xT[t * P : (t + 1) * P, :])
                wp_t = wp_pool.tile([P, C], BF16, tag="wp", name=f"wp{t}")
                nc.sync.dma_start(wp_t, wpT[t * P : (t + 1) * P, :])
                wps.append(wp_t)
        else:
            qk_tiles = emit_qk(0)
            if _DBG:
                nc.sync.dma_start(dbg["qk"][0:P, :], qk_tiles[0])
                nc.sync.dma_start(dbg["qk"][P : 2 * P, :], qk_tiles[1])
            units = []  # queue of (unit, e_t) awaiting PV
            for p in range(NP):
                nqk, ngen = None, None
                # last pair: nt1 units first so their normalize (and the proj
                # tiles reading those aot columns) unblocks earliest
                slots = (
                    [(1, 0), (1, 1), (0, 0), (0, 1)]
                    if p == NP - 1
                    else [(0, 0), (0, 1), (1, 0), (1, 1)]
                )
                for s, (nt, j) in enumerate(slots):
                    u = (p, j, nt, qk_tiles[0], qk_tiles[1])
                    if p == 0:
                        e_t = emit_s_exp(u)
                        # v projection rides between the first pair's S units;
                        # PV must wait until every v tile exists.
                        emit_v_mt(2 * s)
                        emit_v_mt(2 * s + 1)
                    else:
                        e_t = emit_s_exp(u, pv_item=units.pop(0), norm_cb=emit_norm)
                        if len(units) > 2:  # drain the pair-0 backlog early
                            emit_norm()
                            emit_pv(*units.pop(0))
                    if _DBG and p == 0 and j == 0 and nt == 0:
                        nc.sync.dma_start(dbg["e"], e_t)
                    units.append((u, e_t))
                    if s == 0 and p >= 1 and p + 1 < NP:
                        nqk = emit_qk(p + 1)
                if p == 0:
                    nqk = emit_qk(1)
                if p == NP - 2:
                    for t in range(NCH):
                        wp_t = wp_pool.tile([P, C], BF16, tag="wp", name=f"wp{t}")
                        nc.sync.dma_start(wp_t, wpT[t * P : (t + 1) * P, :])
                        wps.append(wp_t)
                if nqk is not None:
                    qk_tiles = nqk
            # drain the last pair's PVs; once a pair-7 nt-group is normalized,
            # the proj tiles reading those aot columns can be emitted
            projed = set()

            def maybe_proj(done):
                if "pproj" in _SKIP or done is None or done[0] != NP - 1:
                    return
                for nt2 in range(done[1] * 4, done[1] * 4 + 4):
                    if nt2 not in projed:
                        projed.add(nt2)
                        emit_proj_nt2(nt2)

            while units:
                flush_dens()
                maybe_proj(emit_norm())
                emit_pv(*units.pop(0))
            flush_dens()
            while pending_norms:
                maybe_proj(emit_norm())
            if "pproj" not in _SKIP:
                for nt2 in range(NMT):
                    if nt2 not in projed:
                        emit_proj_nt2(nt2)
        if _DBG:
            for t in range(NCH):
                nc.sync.dma_start(dbg["aot"][t * P : (t + 1) * P, :], aot[t])
            nc.sync.dma_start(dbg["v"], vsb[0])
        if "pattn" in _SKIP and "pproj" not in _SKIP:
            for nt2 in range(NMT):
                emit_proj_nt2(nt2)

    # Pin the one activation table that serves every ACT func we use
    # (Exp, Ln, Copy all live in 'natural_log_exp_and_others'); the default
    # first-fit table choice alternates tables between the attention exps
    # and the ln/exp denominator ops, costing a ~1.3us table load each time.
    _orig_tables = bacc.get_activation_tables
    def _pinned(arch):
        t = _orig_tables(arch)
        if "natural_log_exp_and_others" not in t:
            return t
        shared = t["natural_log_exp_and_others"]
        out = {}
        for k, fns in t.items():
            if k == "natural_log_exp_and_others":
                out[k] = fns
            else:
                out[k] = fns - shared  # force Exp/Ln/Copy onto the shared table
        return out
    bacc.get_activation_tables = _pinned
    try:
        nc.compile()
    finally:
        bacc.get_activation_tables = _orig_tables
    return nc


def make_in_maps(x, w_qkv, w_proj, b_proj):
    import ml_dtypes

    bf = ml_dtypes.bfloat16
    wqkT = np.ascontiguousarray(w_qkv[: 2 * C].T.astype(bf))
    wqk0 = np.ascontiguousarray(
        np.concatenate([wqkT[:, 0:P], wqkT[:, C : C + P]], axis=1)
    )
    wvT = np.ascontiguousarray(w_qkv[2 * C :].T.astype(bf))
    wpT = np.ascontiguousarray(w_proj.T.astype(bf))
    bias_bc = np.ascontiguousarray(np.broadcast_to(b_proj, (P, C)).astype(np.float32))
    ones = np.ones((P, H), dtype=bf)
    obd = np.zeros((P, P), dtype=bf)
    obd[0, :HD] = 1
    obd[HD, HD:] = 1
    in_maps = []
    for b in range(B):
        in_maps.append(
            {
                "xT": np.ascontiguousarray(x[b].T.astype(bf)),
                "wqkT": wqkT,
                "wqk0": wqk0,
                "wvT": wvT,
                "wpT": wpT,
                "bias_bc": bias_bc,
                "ones_col": ones,
                "ones_bd": obd,
            }
        )
    return in_maps


_CACHED_NC = None


def kernel(x, w_qkv, w_proj, b_proj):
    global _CACHED_NC
    x = np.asarray(x, dtype=np.float32)
    w_qkv = np.asarray(w_qkv, dtype=np.float32)
    w_proj = np.asarray(w_proj, dtype=np.float32)
    b_proj = np.asarray(b_proj, dtype=np.float32)
    if _CACHED_NC is None:
        _CACHED_NC = build_module()
    nc = _CACHED_NC
    in_maps = make_in_maps(x, w_qkv, w_proj, b_proj)
    res = bass_utils.run_bass_kernel_spmd(nc, in_maps, core_ids=list(range(B)))
    return np.stack([res.results[b]["out"] for b in range(B)], axis=0)


if __name__ == "__main__":
    nc = build_module()
    ninst = sum(len(b.instructions) for b in nc.m.functions[0].blocks)
    print("module built ok;", ninst, "instructions")


# revision 87
# speedup vs baseline: 1.1778x; 1.1778x over previous
"""Multi-head attention (B=8, N=1024, C=1024, H=16) on 8 Trainium2 NeuronCores.

Sharding: pure data-parallel — one batch element per core, weights replicated,
no collectives.

v2: all matmuls in bf16 (fp32 matmuls are power-throttled to ~half rate on
TRN2; bf16 runs the PE at full 2.4 GHz). Everything stays SBUF-resident (no
DRAM bounce for qkT). Fused per-head-pair schedule so the ACT engine (exp)
overlaps the projection matmuls. Accumulation is fp32 in PSUM throughout.

Per-core algorithm:
  qk proj   qkT[d, n] = wqkT[c, d].T @ xT[c, n] per head pair, cast bf16.
  v proj    v[m, d] natural layout, interleaved per m-tile as [m, 16*(64+1)]
            with a ones column per head (PV then emits softmax denominators
            for free in PSUM row 64).
  attention per unit (head, nt-half):
            S^T[m, n] = k.T @ q      (keys on partitions, K=64 row-packed)
            E = exp(SCALE * S^T)     (ACT, PSUM -> SBUF bf16)
            U[65, n] = v_aug.T @ E   (row 64 = denominator)
            rden = reciprocal(U[64]) (DVE, fp32)  -> cast bf16
            bc[128, n] = ones_bd.T @ rden   (PE broadcast across partitions)
            aot[c, n] = U[0:64] * bc        (Pool engine, writes bf16)
  proj      out[n, d] = aot[c, n].T @ wpT[c, d] + bias (Pool add), DMA out.
"""

import sys

if "/opt/trn_rl_repo" not in sys.path:
    sys.path.insert(0, "/opt/trn_rl_repo")

from contextlib import ExitStack

import numpy as np

import concourse.bass as bass
import concourse.mybir as mybir
from concourse import bacc
import concourse.tile as tile
from concourse import bass_utils

B, N, C, H = 8, 1024, 1024, 16
HD = C // H          # 64
NP = H // 2          # 8 head pairs
SCALE = HD ** -0.5   # 0.125
P = 128              # SBUF partitions
NT = 512             # psum-bank moving tile
NCH = C // P         # 8 contraction chunks over channels
NMT = N // P         # 8 token tiles of 128
NNT = N // NT        # 2 token tiles of 512
F32 = mybir.dt.float32
BF16 = mybir.dt.bfloat16
EXP = mybir.ActivationFunctionType.Exp


def build_module():
    import os
    _SKIP = set(filter(None, os.environ.get("K_SKIP", "").split(",")))
    nc = bacc.Bacc("TRN2", target_bir_lowering=False, debug=False, num_devices=B)

    xT = nc.dram_tensor("xT", [C, N], BF16, kind="ExternalInput").ap()
    wqkT = nc.dram_tensor("wqkT", [C, 2 * C], BF16, kind="ExternalInput").ap()
    # pair-0's q/k weight columns, duplicated for a fast startup path
    wqk0 = nc.dram_tensor("wqk0", [C, 2 * P], BF16, kind="ExternalInput").ap()
    wvT = nc.dram_tensor("wvT", [C, C], BF16, kind="ExternalInput").ap()
    wpT = nc.dram_tensor("wpT", [C, C], BF16, kind="ExternalInput").ap()
    bias = nc.dram_tensor("bias_bc", [P, C], F32, kind="ExternalInput").ap()
    ones_col = nc.dram_tensor("ones_col", [P, H], BF16, kind="ExternalInput").ap()
    ones_bd = nc.dram_tensor("ones_bd", [P, P], BF16, kind="ExternalInput").ap()
    out = nc.dram_tensor("out", [N, C], F32, kind="ExternalOutput").ap()
    _DBG = os.environ.get("K_DEBUG", "")
    dbg = {}
    if _DBG:
        dbg["aot"] = nc.dram_tensor("dbg_aot", [C, N], BF16, kind="ExternalOutput").ap()
        dbg["qk"] = nc.dram_tensor("dbg_qk", [2 * P, N], BF16, kind="ExternalOutput").ap()
        dbg["v"] = nc.dram_tensor("dbg_v", [P, H * (HD + 1)], BF16, kind="ExternalOutput").ap()
        dbg["e"] = nc.dram_tensor("dbg_e", [P, N * NMT // 2], BF16, kind="ExternalOutput").ap()
        dbg["rb"] = nc.dram_tensor("dbg_rb", [P, NT], BF16, kind="ExternalOutput").ap()
        dbg["rc"] = nc.dram_tensor("dbg_rc", [P, NT], F32, kind="ExternalOutput").ap()

    with tile.TileContext(nc) as tc, ExitStack() as ctx:
        xt_pool = ctx.enter_context(tc.tile_pool(name="xt", bufs=NCH))
        wqk_pool = ctx.enter_context(tc.tile_pool(name="wqk", bufs=NCH))
        wv_pool = ctx.enter_context(tc.tile_pool(name="wv", bufs=NCH))
        wp_pool = ctx.enter_context(tc.tile_pool(name="wp", bufs=NCH))
        qk_pool = ctx.enter_context(tc.tile_pool(name="qk", bufs=3))
        vsb_pool = ctx.enter_context(tc.tile_pool(name="vsb", bufs=1))
        e_pool = ctx.enter_context(tc.tile_pool(name="e", bufs=5))
        aot_pool = ctx.enter_context(tc.tile_pool(name="aot", bufs=1))
        one_pool = ctx.enter_context(tc.tile_pool(name="one", bufs=1))
        rden_pool = ctx.enter_context(tc.tile_pool(name="rden", bufs=2))
        osb_pool = ctx.enter_context(tc.tile_pool(name="osb", bufs=2))
        ps_pool = ctx.enter_context(tc.tile_pool(name="ps", bufs=2, space="PSUM"))
        ps2_pool = ctx.enter_context(tc.tile_pool(name="ps2", bufs=1, space="PSUM"))
        pu_pool = ctx.enter_context(tc.tile_pool(name="pu", bufs=3, space="PSUM"))

        # ---------- input loads ----------
        # pair-0 weight slices ride interleaved with x so qk(0) is paced by
        # the x tiles, not the full 4MB wqk load
        # The pair-0 weight fast path (wqk0) shaved ~6us of startup but showed
        # rare first-run corruption on hardware (cold-DMA race); disabled.
        _NOWQK0 = not os.environ.get("K_WQK0")
        xts, wvs, wqks, wqk0s = [], [], [], []
        for t in range(NCH):
            xt_t = xt_pool.tile([P, N], BF16, tag="xt", name=f"xt{t}")
            nc.sync.dma_start(xt_t, xT[t * P : (t + 1) * P, :])
            xts.append(xt_t)
            if not _NOWQK0:
                w0_t = wqk_pool.tile([P, 2 * P], BF16, tag="wqk0", name=f"wqk0_{t}")
                nc.sync.dma_start(w0_t, wqk0[t * P : (t + 1) * P, :])
                wqk0s.append(w0_t)
        for t in range(NCH):
            wqk_t = wqk_pool.tile([P, 2 * C], BF16, tag="wqk", name=f"wqk{t}")
            nc.sync.dma_start(wqk_t, wqkT[t * P : (t + 1) * P, :])
            wqks.append(wqk_t)
        for t in range(NCH):
            wv_t = wv_pool.tile([P, C], BF16, tag="wv", name=f"wv{t}")
            nc.sync.dma_start(wv_t, wvT[t * P : (t + 1) * P, :])
            wvs.append(wv_t)
        bias_sb = one_pool.tile([P, C], F32, tag="bias", name="bias_sb")
        nc.sync.dma_start(bias_sb, bias)
        onesbd_sb = one_pool.tile([P, P], BF16, tag="obd", name="onesbd_sb")
        nc.sync.dma_start(onesbd_sb, ones_bd)

        # persistent zero-padded k tiles and denominator-broadcast sources:
        # zeroed once here; later writers only touch the data quadrants /
        # rows 0 and 64, so the zero regions stay valid across reuse
        kpads = []
        for i in range(3):
            kp_t = qk_pool.tile([P, 2 * N], BF16, tag="kpad", name=f"kpad{i}")
            nc.gpsimd.memset(kp_t, 0.0)
            kpads.append(kp_t)
        rbs = []
        for i in range(2):
            rb_t = rden_pool.tile([P, NT], BF16, tag="rb", name=f"rb{i}")
            nc.gpsimd.memset(rb_t, 0.0)
            rbs.append(rb_t)

        # v tiles (natural layout + ones cols), attention-out accumulators
        vsb = []
        for mt in range(NMT):
            v_t = vsb_pool.tile([P, H * (HD + 1)], BF16, tag=f"v{mt}", name=f"v{mt}")
            nc.sync.dma_start(
                v_t.rearrange("p (h w) -> p h w", w=HD + 1)[:, :, HD : HD + 1], ones_col
            )
            vsb.append(v_t)
        aot = []
        for t in range(NCH):
            a_t = aot_pool.tile([P, N], BF16, tag=f"aot{t}", name=f"aot{t}")
            aot.append(a_t)

        # ---------- phase emitters ----------
        def emit_v_mt(mt):
            """v projection for one m-tile: psum [128, 1024] = both d halves."""
            ps_v = ps_pool.tile([P, N], F32, tag="ps", name=f"psv{mt}")
            for ck in range(NCH):
                for hv in range(2):
                    nc.tensor.matmul(
                        ps_v[:, hv * NT : (hv + 1) * NT],
                        lhsT=xts[ck][:, mt * P : (mt + 1) * P],
                        rhs=wvs[ck][:, hv * NT : (hv + 1) * NT],
                        start=(ck == 0),
                        stop=(ck == NCH - 1),
                    )
            dst = vsb[mt].rearrange("p (h w) -> p h w", w=HD + 1)[:, :, 0:HD]
            nc.vector.tensor_copy(dst, ps_v.rearrange("p (h w) -> p h w", w=HD))

        def emit_qk_chunks(p):
            """qk projection for head pair p, as a generator yielding after
            each (tensor, nt-half) chunk so the chunks can be spread across
            the previous pair's unit slots (keeps the PE fed while the ACT
            engine drains exps). q -> bf16 [128, 1024]; k -> a zero-padded
            [128, 2048] (head A in rows 0:64 of cols 0:1024, head B in rows
            64:128 of cols 1024:2048, zeros elsewhere) so the S matmuls
            contract over K=128 — K=64 matmuls run ~1.8x slower on the PE."""
            q_sb = qk_pool.tile([P, N], BF16, tag="qk", name=f"qk{p}")
            kpad = kpads[p % 3]

            def gen():
                for which in range(2):  # 0 = q rows, 1 = k rows
                    dlo = which * C + p * P
                    sb = q_sb if which == 0 else kpad
                    for nt in range(NNT):
                        ps_qk = ps2_pool.tile(
                            [P, NT], F32, tag="ps2", name=f"psqk{p}_{which}_{nt}"
                        )
                        for ck in range(NCH):
                            lhsT = (
                                wqk0s[ck][:, which * P : (which + 1) * P]
                                if p == 0 and wqk0s
                                else wqks[ck][:, dlo : dlo + P]
                            )
                            nc.tensor.matmul(
                                ps_qk,
                                lhsT=lhsT,
                                rhs=xts[ck][:, nt * NT : (nt + 1) * NT],
                                start=(ck == 0),
                                stop=(ck == NCH - 1),
                            )
                        if which == 0:
                            nc.vector.tensor_copy(sb[:, nt * NT : (nt + 1) * NT], ps_qk)
                        else:
                            nc.vector.tensor_copy(
                                sb[0:HD, nt * NT : (nt + 1) * NT], ps_qk[0:HD, :]
                            )
                            nc.vector.tensor_copy(
                                sb[HD:P, N + nt * NT : N + (nt + 1) * NT],
                                ps_qk[HD:P, :],
                            )
                        yield

            return [q_sb, kpad], gen()

        def emit_qk(p):
            tiles, g = emit_qk_chunks(p)
            for _ in g:
                pass
            return tiles

        def emit_s_exp(u, pv_item=None, norm_cb=None):
            """S^T matmuls + exp for one unit (pair p, head-slot j, nt).
            The S block is split in two halves with the previous unit's PV
            block (and the pending normalize) between them, so the PE has a
            block of independent matmuls to chew while the exps of the first
            half drain their psum tiles. Contracts over all 128 partitions;
            the kpad zero rows mask out the other head."""
            (p, j, nt, qt, kt) = u
            e_t = e_pool.tile([P, N * NMT // 2], BF16, tag="e", name=f"e{p}_{j}_{nt}")

            def s_half(g0):
                for g in range(g0, g0 + 2):  # two m-chunks per psum tile
                    ps_s = ps_pool.tile([P, N], F32, tag="ps", name=f"pss{p}_{j}_{nt}_{g}")
                    for half in range(2):
                        mc = 2 * g + half
                        nc.tensor.matmul(
                            ps_s[:, half * NT : (half + 1) * NT],
                            lhsT=kt[:, j * N + mc * P : j * N + (mc + 1) * P],
                            rhs=qt[:, nt * NT : (nt + 1) * NT],
                            start=True,
                            stop=True,
                        )
                    nc.scalar.activation(
                        e_t[:, g * N : (g + 1) * N], ps_s, EXP, scale=SCALE
                    )

            s_half(0)
            s_half(2)
            if norm_cb is not None:
                norm_cb()
            if pv_item is not None:
                emit_pv(*pv_item)
            return e_t

        pair_nt_state = {}
        pending_norms = []
        pending_dens = []

        def flush_dens():
            while pending_dens:
                emit_pv_den(*pending_dens.pop(0))

        def emit_pv_mms(u, e_t):
            """Generator: PV matmuls two at a time (yield between batches),
            then the denominator ln/exp chain."""
            (p, j, nt, qt, kt) = u
            h = 2 * p + j
            ps_u = pu_pool.tile([HD + 1, NT], F32, tag="pu", name=f"psu{h}_{nt}")
            for mc in range(NMT):
                nc.tensor.matmul(
                    ps_u,
                    lhsT=vsb[mc][:, h * (HD + 1) : (h + 1) * (HD + 1)],
                    rhs=e_t[:, mc * NT : (mc + 1) * NT],
                    start=(mc == 0),
                    stop=(mc == NMT - 1),
                )
                if mc % 2 == 1 and mc < NMT - 1:
                    yield
            emit_pv_den(u, ps_u)
            yield

        def emit_pv(u, e_t):
            for _ in emit_pv_mms(u, e_t):
                pass

        def emit_pv_den(u, ps_u):
            """Denominator 1/den = exp(-ln(den)) on ACT (same act table
            as the attention exps -> no table switches; a DVE exact
            reciprocal was tried instead and regressed 48us — its 4us serial
            op blocks DVE's in-order queue ahead of the latency-critical qk
            casts). The broadcast matmul + normalize is queued and emitted a
            slot later so the PE never stalls in-order on the ACT chain."""
            (p, j, nt, qt, kt) = u
            key = (p, nt)
            if key not in pair_nt_state:
                rb = rbs[(2 * p + nt) % 2]
                lt = rden_pool.tile([1, 2 * NT], F32, tag="lt", name=f"lt{p}_{nt}")
                pair_nt_state[key] = {"rb": rb, "lt": lt, "us": []}
            st = pair_nt_state[key]
            LN = mybir.ActivationFunctionType.Ln
            nc.scalar.activation(
                st["lt"][:, j * NT : (j + 1) * NT], ps_u[HD : HD + 1, :], LN
            )
            nc.scalar.activation(
                st["rb"][j * HD : j * HD + 1, :],
                st["lt"][:, j * NT : (j + 1) * NT],
                EXP,
                scale=-1.0,
            )
            st["us"].append((j, ps_u))
            if len(st["us"]) == 2:
                pair_nt_state.pop(key)
                pending_norms.append((p, nt, st))

        def emit_norm():
            if not pending_norms:
                return None
            (p, nt, st) = pending_norms.pop(0)
            bc = ps2_pool.tile([P, NT], F32, tag="ps2", name=f"bc{p}_{nt}")
            nc.tensor.matmul(
                bc,
                lhsT=onesbd_sb[0 : HD + 1, :],
                rhs=st["rb"][0 : HD + 1, :],
                start=True,
                stop=True,
            )
            rbc = rden_pool.tile([HD, 2 * NT], F32, tag="rc", name=f"rc{p}_{nt}")
            nc.vector.tensor_copy(rbc[:, 0:NT], bc[0:HD, :])
            nc.vector.tensor_copy(rbc[:, NT : 2 * NT], bc[HD : 2 * HD, :])
            if _DBG and p == 0 and nt == 0:
                nc.sync.dma_start(dbg["rb"], st["rb"])
                nc.sync.dma_start(dbg["rc"][0:HD, :], rbc[:, 0:NT])
            for (jj, psu) in st["us"]:
                nc.vector.tensor_mul(
                    aot[p][jj * HD : (jj + 1) * HD, nt * NT : (nt + 1) * NT],
                    psu[0:HD, :],
                    rbc[:, jj * NT : (jj + 1) * NT],
                )
            return (p, nt)

        def emit_proj_nt2(nt2):
            ps_o = ps_pool.tile([P, C], F32, tag="ps", name=f"pso{nt2}")
            for ck in range(NCH):
                for dt in range(2):
                    nc.tensor.matmul(
                        ps_o[:, dt * NT : (dt + 1) * NT],
                        lhsT=aot[ck][:, nt2 * P : (nt2 + 1) * P],
                        rhs=wps[ck][:, dt * NT : (dt + 1) * NT],
                        start=(ck == 0),
                        stop=(ck == NCH - 1),
                    )
            o_sb = osb_pool.tile([P, C], F32, tag="osb", name=f"o{nt2}")
            nc.vector.tensor_add(o_sb, ps_o, bias_sb)
            nc.sync.dma_start(out[nt2 * P : (nt2 + 1) * P, :], o_sb)

        # ---------- fused schedule ----------
        wps = []
        if "pattn" in _SKIP:
            for t in range(NCH):
                nc.sync.dma_start(aot[t], xT[t * P : (t + 1) * P, :])
                wp_t = wp_pool.tile([P, C], BF16, tag="wp", name=f"wp{t}")
                nc.sync.dma_start(wp_t, wpT[t * P : (t + 1) * P, :])
                wps.append(wp_t)
        else:
            qk_tiles = emit_qk(0)
            if _DBG:
                nc.sync.dma_start(dbg["qk"][0:P, :], qk_tiles[0])
                nc.sync.dma_start(dbg["qk"][P : 2 * P, :], qk_tiles[1])
            units = []  # queue of (unit, e_t) awaiting PV
            for p in range(NP):
                nqk, ngen = None, None
                # last pair: nt1 units first so their normalize (and the proj
                # tiles reading those aot columns) unblocks earliest
                slots = (
                    [(1, 0), (1, 1), (0, 0), (0, 1)]
                    if p == NP - 1
                    else [(0, 0), (0, 1), (1, 0), (1, 1)]
                )
                for s, (nt, j) in enumerate(slots):
                    u = (p, j, nt, qk_tiles[0], qk_tiles[1])
                    if p == 0:
                        e_t = emit_s_exp(u)
                        # v projection rides between the first pair's S units;
                        # PV must wait until every v tile exists.
                        emit_v_mt(2 * s)
                        emit_v_mt(2 * s + 1)
                    else:
                        e_t = emit_s_exp(u, pv_item=units.pop(0), norm_cb=emit_norm)
                        if len(units) > 2:  # drain the pair-0 backlog early
                            emit_norm()
                            emit_pv(*units.pop(0))
                    if _DBG and p == 0 and j == 0 and nt == 0:
                        nc.sync.dma_start(dbg["e"], e_t)
                    units.append((u, e_t))
                    if s == 0 and p >= 1 and p + 1 < NP:
                        nqk = emit_qk(p + 1)
                if p == 0:
                    nqk = emit_qk(1)
                if p == NP - 2:
                    for t in range(NCH):
                        wp_t = wp_pool.tile([P, C], BF16, tag="wp", name=f"wp{t}")
                        nc.sync.dma_start(wp_t, wpT[t * P : (t + 1) * P, :])
                        wps.append(wp_t)
                if nqk is not None:
                    qk_tiles = nqk
            # drain the last pair's PVs; once a pair-7 nt-group is normalized,
            # the proj tiles reading those aot columns can be emitted
            projed = set()

            def maybe_proj(done):
                if "pproj" in _SKIP or done is None or done[0] != NP - 1:
                    return
                for nt2 in range(done[1] * 4, done[1] * 4 + 4):
                    if nt2 not in projed:
                        projed.add(nt2)
                        emit_proj_nt2(nt2)

            while units:
                flush_dens()
                maybe_proj(emit_norm())
                emit_pv(*units.pop(0))
            flush_dens()
            while pending_norms:
                maybe_proj(emit_norm())
            if "pproj" not in _SKIP:
                for nt2 in range(NMT):
                    if nt2 not in projed:
                        emit_proj_nt2(nt2)
        if _DBG:
            for t in range(NCH):
                nc.sync.dma_start(dbg["aot"][t * P : (t + 1) * P, :], aot[t])
            nc.sync.dma_start(dbg["v"], vsb[0])
        if "pattn" in _SKIP and "pproj" not in _SKIP:
            for nt2 in range(NMT):
                emit_proj_nt2(nt2)

    # Pin the one activation table that serves every ACT func we use
    # (Exp, Ln, Copy all live in 'natural_log_exp_and_others'); the default
    # first-fit table choice alternates tables between the attention exps
    # and the ln/exp denominator ops, costing a ~1.3us table load each time.
    _orig_tables = bacc.get_activation_tables
    def _pinned(arch):
        t = _orig_tables(arch)
        if "natural_log_exp_and_others" not in t:
            return t
        shared = t["natural_log_exp_and_others"]
        out = {}
        for k, fns in t.items():
            if k == "natural_log_exp_and_others":
                out[k] = fns
            else:
                out[k] = fns - shared  # force Exp/Ln/Copy onto the shared table
        return out
    bacc.get_activation_tables = _pinned
    try:
        nc.compile()
    finally:
        bacc.get_activation_tables = _orig_tables
    return nc


def make_in_maps(x, w_qkv, w_proj, b_proj):
    import ml_dtypes

    bf = ml_dtypes.bfloat16
    wqkT = np.ascontiguousarray(w_qkv[: 2 * C].T.astype(bf))
    wqk0 = np.ascontiguousarray(
        np.concatenate([wqkT[:, 0:P], wqkT[:, C : C + P]], axis=1)
    )
    wvT = np.ascontiguousarray(w_qkv[2 * C :].T.astype(bf))
    wpT = np.ascontiguousarray(w_proj.T.astype(bf))
    bias_bc = np.ascontiguousarray(np.broadcast_to(b_proj, (P, C)).astype(np.float32))
    ones = np.ones((P, H), dtype=bf)
    obd = np.zeros((P, P), dtype=bf)
    obd[0, :HD] = 1
    obd[HD, HD:] = 1
    in_maps = []
    for b in range(B):
        in_maps.append(
            {
                "xT": np.ascontiguousarray(x[b].T.astype(bf)),
                "wqkT": wqkT,
                "wqk0": wqk0,
                "wvT": wvT,
                "wpT": wpT,
                "bias_bc": bias_bc,
                "ones_col": ones,
                "ones_bd": obd,
            }
        )
    return in_maps


_CACHED_NC = None


def kernel(x, w_qkv, w_proj, b_proj):
    global _CACHED_NC
    x = np.asarray(x, dtype=np.float32)
    w_qkv = np.asarray(w_qkv, dtype=np.float32)
    w_proj = np.asarray(w_proj, dtype=np.float32)
    b_proj = np.asarray(b_proj, dtype=np.float32)
    if _CACHED_NC is None:
        _CACHED_NC = build_module()
    nc = _CACHED_NC
    in_maps = make_in_maps(x, w_qkv, w_proj, b_proj)
    res = bass_utils.run_bass_kernel_spmd(nc, in_maps, core_ids=list(range(B)))
    return np.stack([res.results[b]["out"] for b in range(B)], axis=0)


if __name__ == "__main__":
    nc = build_module()
    ninst = sum(len(b.instructions) for b in nc.m.functions[0].blocks)
    print("module built ok;", ninst, "instructions")
